# revision 16
# baseline (speedup 1.0000x reference)
"""Known-good E2 kernel (HW exec 137766 ns, rel err 0.0049).

Restore with: cp /tmp/kernel_e2_good.py /root/problem/kernel.py
All-u8 stores; u8 cvt split 11 ACT / 5 DVE; NG=4; baseline-style sems.
"""

import os
import sys

import numpy as np

sys.path.insert(0, "/opt/trn_rl_repo")

BATCH = 4096
IN_DIM = 8192
OUT_DIM = 16384
N_CORES = 8
J_SHARD = OUT_DIM // N_CORES
CHUNK = 128
N_CHUNKS = J_SHARD // CHUNK
NG = 4
NT = 2
NH = 3
NO = 4
NO8 = 6
CLAG = 2
DVE_PRE = 14

GATE_COEFFS = np.array([
    [0, 0, 0, 0], [0, 0, 0, 1], [0, 1, 0, -1], [0, 1, 0, 0],
    [0, 0, 1, -1], [0, 0, 1, 0], [0, 1, 1, -2], [0, 1, 1, -1],
    [1, -1, -1, 1], [1, -1, -1, 2], [1, 0, -1, 0], [1, 0, -1, 1],
    [1, -1, 0, 0], [1, -1, 0, 1], [1, 0, 0, -1], [1, 0, 0, 0],
], dtype=np.float32)

OSCALE = 254.0


def cvt_owner(j):
    return 'dve' if j % 3 == 2 else 'act'


_CACHE = {}
LAST_RESULT = None


def _wrap_idx16(idx_pair):
    cols = []
    for c in range(N_CHUNKS):
        merged = np.concatenate([idx_pair[0, c * CHUNK:(c + 1) * CHUNK],
                                 idx_pair[1, c * CHUNK:(c + 1) * CHUNK]])
        cols.append(merged.astype(np.int16).reshape(16, 16).T)
    blk = np.concatenate(cols, axis=1)
    return np.ascontiguousarray(np.tile(blk, (8, 1)))


def _build_program():
    import concourse.bacc as bacc
    import concourse.mybir as mybir
    from concourse.library_config import mlp
    from contextlib import ExitStack

    dt = mybir.dt
    AF = mybir.ActivationFunctionType

    nc = bacc.Bacc("TRN2", target_bir_lowering=False, debug=False)

    xt = nc.dram_tensor("xt", [IN_DIM, BATCH], dt.float16, kind="ExternalInput")
    idx = nc.dram_tensor("idx", [128, 2 * (J_SHARD // 16)], dt.int16,
                         kind="ExternalInput")
    wgt = nc.dram_tensor("wgt", [128, N_CHUNKS * 16], dt.float32,
                         kind="ExternalInput")
    gcr = nc.dram_tensor("gcr", [128, 4 * N_CHUNKS * 16], dt.float32,
                         kind="ExternalInput")
    out = nc.dram_tensor("out", [J_SHARD, BATCH], dt.uint8,
                         kind="ExternalOutput")

    W16 = N_CHUNKS * 16

    with ExitStack() as ctx:
        sb = lambda name, shape, dty: ctx.enter_context(
            nc.sbuf_tensor(name, shape, dty))
        sb_idx = sb("sb_idx", [128, 2 * (J_SHARD // 16)], dt.int16)
        sb_w = sb("sb_w", [128, W16], dt.float32)
        sb_gc = sb("sb_gc", [128, 4 * W16], dt.float32)
        sb_e = sb("sb_e", [128, W16], dt.float32)
        sb_scr = sb("sb_scr", [128, W16], dt.float32)
        sb_s = sb("sb_s", [128, N_CHUNKS], dt.float32)
        sb_r = sb("sb_r", [128, N_CHUNKS], dt.float32)
        sb_cc = sb("sb_cc", [128, 4 * N_CHUNKS], dt.float32)
        ab_bufs = [sb(f"ab{k}", [128, 2, BATCH], dt.float16) for k in range(NG)]
        t_bufs = [sb(f"t{k}", [128, BATCH], dt.float16) for k in range(NT)]
        h_bufs = [sb(f"h{k}", [128, BATCH], dt.float16) for k in range(NH)]
        o_bufs = [sb(f"o{k}", [128, BATCH], dt.float16) for k in range(NO)]
        o8_bufs = [sb(f"q{k}", [128, BATCH], dt.uint8) for k in range(NO8)]

        ops_act = []
        for i in range(N_CHUNKS):
            ops_act.append(('h', i))
            j = i - CLAG
            if j >= 0 and cvt_owner(j) == 'act':
                ops_act.append(('cvt', j))
        for j in range(N_CHUNKS - CLAG, N_CHUNKS):
            if cvt_owner(j) == 'act':
                ops_act.append(('cvt', j))
        act_val = {op: n + 1 for n, op in enumerate(ops_act)}

        ops_dve = []
        for i in range(N_CHUNKS):
            ops_dve.append(('ts', i))
            if i > 0:
                ops_dve.append(('add', i - 1))
            j = i - CLAG
            if j >= 0 and cvt_owner(j) == 'dve':
                ops_dve.append(('cvt', j))
            ops_dve.append(('mul', i))
        ops_dve.append(('add', N_CHUNKS - 1))
        for j in range(N_CHUNKS - CLAG, N_CHUNKS):
            if cvt_owner(j) == 'dve':
                ops_dve.append(('cvt', j))
        # ts ops don't inc s_dve (no cross-engine consumer — same-engine
        # RAWs are ordered by in-order execution); only mul/add/cvt count.
        dve_val = {}
        _n = DVE_PRE
        for op in ops_dve:
            if op[0] != 'ts':
                _n += 1
                dve_val[op] = _n

        def cvt_done_wait(eng, j):
            if cvt_owner(j) == 'act':
                eng.wait_ge(s_act, act_val[('cvt', j)])
            else:
                eng.wait_ge(s_dve, dve_val[('cvt', j)])

        with (
            nc.Block() as block,
            nc.semaphore("s_pi") as s_pi,
            nc.semaphore("s_pw") as s_pw,
            nc.semaphore("s_pg") as s_pg,
            nc.semaphore("s_exp") as s_exp,
            nc.semaphore("s_g0") as s_g0,
            nc.semaphore("s_g1") as s_g1,
            nc.semaphore("s_g2") as s_g2,
            nc.semaphore("s_g3") as s_g3,
            nc.semaphore("s_st0") as s_st0,
            nc.semaphore("s_st1") as s_st1,
            nc.semaphore("s_st2") as s_st2,
            nc.semaphore("s_st3") as s_st3,
            nc.semaphore("s_st4") as s_st4,
            nc.semaphore("s_st5") as s_st5,
            nc.semaphore("s_act") as s_act,
            nc.semaphore("s_dve") as s_dve,
        ):
            s_g = [s_g0, s_g1, s_g2, s_g3]
            s_st = [s_st0, s_st1, s_st2, s_st3, s_st4, s_st5]

            def cseg(k, i):
                return sb_cc[:, 16 * k + i : 16 * k + i + 1]

            @block.sync
            def _(sync):
                sync.dma_start(sb_idx[:, :], idx[:, :]).then_inc(s_pi, 16)
                sync.dma_start(sb_w[:, :], wgt[:, :]).then_inc(s_pw, 16)
                sync.dma_start(sb_gc[:, :], gcr[:, :]).then_inc(s_pg, 16)
                for i in range(N_CHUNKS):
                    ks = i % NO8
                    cvt_done_wait(sync, i)
                    sync.dma_start(out[i * CHUNK:(i + 1) * CHUNK, :],
                                   o8_bufs[ks][:, :]).then_inc(s_st[ks], 16)
                for ks in range(NO8):
                    n_st = (N_CHUNKS - 1 - ks) // NO8 + 1
                    sync.wait_ge(s_st[ks], 16 * n_st)

            @block.gpsimd
            def _(gp):
                gp.load_library(mlp)
                nreg = gp.alloc_register("nidx")
                gp.reg_mov(nreg, 2 * CHUNK)
                gp.wait_ge(s_pi, 16)
                for i in range(N_CHUNKS):
                    kg = i % NG
                    if i >= NG:
                        gp.wait_ge(s_dve, dve_val[('mul', i - NG)])
                        gp.wait_ge(s_act, act_val[('h', i - NG)])
                        gp.wait_ge(s_g[kg], 16 * (i // NG))
                    gp.dma_gather(
                        ab_bufs[kg].ap(), xt.ap(),
                        sb_idx[:, 16 * i:16 * i + 16], 2 * CHUNK, nreg, BATCH,
                    ).then_inc(s_g[kg], 16)

            @block.scalar
            def _(sc):
                sc.wait_ge(s_pw, 16)
                sc.activation(sb_e[:, :], sb_w[:, :], AF.Exp).then_inc(s_exp, 1)
                sc.wait_ge(s_dve, DVE_PRE)
                for kind, i in ops_act:
                    if kind == 'h':
                        k = i % NH
                        kg = i % NG
                        sc.wait_ge(s_g[kg], 16 * (i // NG + 1))
                        if i >= NH:
                            sc.wait_ge(s_dve, dve_val[('add', i - NH)])
                        sc.activation(h_bufs[k][:, :], ab_bufs[kg][:, 1, :],
                                      AF.Identity,
                                      bias=cseg(0, i), scale=cseg(2, i),
                                      ).then_inc(s_act, 1)
                    else:
                        ko, ks = i % NO, i % NO8
                        sc.wait_ge(s_dve, dve_val[('add', i)])
                        if i >= NO8:
                            sc.wait_ge(s_st[ks], 16 * (i // NO8))
                        sc.activation(o8_bufs[ks][:, :], o_bufs[ko][:, :],
                                      AF.Identity,
                                      ).then_inc(s_act, 1)

            @block.vector
            def _(v):
                X = mybir.AxisListType.X
                n = 0

                def step(ins):
                    nonlocal n
                    n += 1
                    ins.then_inc(s_dve, 1)

                v.wait_ge(s_exp, 1)
                v.wait_ge(s_pg, 16)
                e3 = sb_e[:, :].rearrange("p (c g) -> p c g", g=16)
                step(v.reduce_sum(sb_s[:, :], e3, axis=X))
                v.wait_ge(s_dve, n)
                step(v.reciprocal(sb_r[:, :], sb_s[:, :]))
                for kk in range(4):
                    if kk > 0:
                        v.wait_ge(s_dve, n)
                    step(v.tensor_mul(sb_scr[:, :], sb_e[:, :],
                                      sb_gc[:, kk * W16:(kk + 1) * W16]))
                    v.wait_ge(s_dve, n)
                    step(v.reduce_sum(
                        sb_cc[:, 16 * kk:16 * (kk + 1)],
                        sb_scr[:, :].rearrange("p (c g) -> p c g", g=16),
                        axis=X))
                v.wait_ge(s_dve, n)
                for kk in range(4):
                    step(v.tensor_mul(sb_cc[:, 16 * kk:16 * (kk + 1)],
                                      sb_cc[:, 16 * kk:16 * (kk + 1)],
                                      sb_r[:, :]))
                assert n == DVE_PRE
                v.wait_ge(s_dve, DVE_PRE)
                MU, AD = mybir.AluOpType.mult, mybir.AluOpType.add
                for kind, i in ops_dve:
                    kt, kh, ko, kg = i % NT, i % NH, i % NO, i % NG
                    ks = i % NO8
                    if kind == 'ts':
                        # t-buf reuse (mul(i-NT)) is same-engine, in-order;
                        # no sem needed, and nothing cross-waits on ts.
                        v.wait_ge(s_g[kg], 16 * (i // NG + 1))
                        v.tensor_scalar(t_bufs[kt][:, :], ab_bufs[kg][:, 1, :],
                                        cseg(3, i), cseg(1, i), MU, AD)
                    elif kind == 'mul':
                        # ts(i) RAW is same-engine, in-order.
                        if i >= NO:
                            j = i - NO
                            if cvt_owner(j) == 'act':
                                v.wait_ge(s_act, act_val[('cvt', j)])
                        v.tensor_mul(o_bufs[ko][:, :], t_bufs[kt][:, :],
                                     ab_bufs[kg][:, 0, :]).then_inc(s_dve, 1)
                    elif kind == 'add':
                        # mul(i) RAW is same-engine, in-order.
                        v.wait_ge(s_act, act_val[('h', i)])
                        v.tensor_add(o_bufs[ko][:, :], o_bufs[ko][:, :],
                                     h_bufs[kh][:, :]).then_inc(s_dve, 1)
                    else:
                        if i >= NO8:
                            v.wait_ge(s_st[ks], 16 * (i // NO8))
                        v.tensor_copy(o8_bufs[ks][:, :],
                                      o_bufs[ko][:, :]).then_inc(s_dve, 1)

    nc.compile()
    return nc


def _get_program():
    if "nc" not in _CACHE:
        _CACHE["nc"] = _build_program()
    return _CACHE["nc"]


def kernel(x, weight, indices):
    global LAST_RESULT
    from concourse.bass_utils import run_bass_kernel_spmd

    x = np.asarray(x, dtype=np.float32)
    weight = np.asarray(weight, dtype=np.float32)
    indices = np.asarray(indices)

    nc = _get_program()

    xt = np.ascontiguousarray(x.T.astype(np.float16))

    gate_scaled = GATE_COEFFS * OSCALE
    gate_scaled[:, 0] += 0.5
    gc_rep = np.broadcast_to(
        gate_scaled.T.reshape(4, 1, 16),
        (4, N_CHUNKS, 16)).reshape(1, -1)
    gc_rep = np.ascontiguousarray(
        np.broadcast_to(gc_rep, (128, 4 * N_CHUNKS * 16)).astype(np.float32))

    in_maps = []
    for c in range(N_CORES):
        j0 = c * J_SHARD
        idx_c = _wrap_idx16(indices[:, j0:j0 + J_SHARD])
        wsh = weight[j0:j0 + J_SHARD]
        w_wrapped = np.ascontiguousarray(
            wsh.reshape(N_CHUNKS, 128, 16).transpose(1, 0, 2)
            .reshape(128, N_CHUNKS * 16))
        in_maps.append({
            "xt": xt,
            "idx": idx_c,
            "wgt": w_wrapped,
            "gcr": gc_rep,
        })

    trace = bool(os.environ.get("KERNEL_TRACE"))
    res = run_bass_kernel_spmd(nc, in_maps, core_ids=list(range(N_CORES)),
                               trace=trace)
    LAST_RESULT = res

    shards = [res.results[c]["out"] for c in range(N_CORES)]
    full = np.concatenate(shards, axis=0)
    return np.ascontiguousarray(full.T.astype(np.float32) * (1.0 / OSCALE))


# revision 26
# speedup vs baseline: 1.1749x; 1.1749x over previous
"""Trainium2 Bass kernel for nn_LogicDense (difflogic dense layer).

Math (reference):
    w      = softmax(weight, axis=-1)            # [out_dim, 16]
    coeffs = w @ GATE_COEFFS                     # [out_dim, 4] = (c0, ca, cb, cab)
    a      = x[:, indices[0]]                    # [batch, out_dim]
    b      = x[:, indices[1]]
    out    = c0 + ca*a + cb*b + cab*a*b          # [batch, out_dim]

Strategy (8 NeuronCores, tensor-parallel over out_dim):
    - Host transposes x -> x_t [in_dim, batch] (fp16, replicated). Core c
      owns output rows j in [2048*c, 2048*(c+1)).
    - Per 128-row chunk: one GPSIMD dma_gather pulls the 256 rows
      x_t[idx0[chunk]] ++ x_t[idx1[chunk]] from HBM into SBUF (row i of
      the index list lands on partition i%128, slot i//128; full batch
      on the free dim). Measured 372 GB/s on the SWDGE ring.
    - Per-partition coeff scalars give the combine (all fp16):
         t  = cab*b + ca     (DVE tensor_scalar, 4x mode)
         h  = cb*b + c0'     (ACT Identity activation, scale/bias APs)
         g  = t * a          (DVE tensor_mul, 2x)
         g  = g + h          (DVE tensor_add, 2x)
         o8 = u8(g)          (ACT Identity for 11/16 chunks, DVE
                              tensor_copy for 5/16 — the split balances
                              both engines at ~100 us each)
    - out is mathematically in [0, 1] (convex combination of gate
      values), so g = OSCALE*out + 0.5 fits u8: the u8 store halves
      store HBM traffic (8 vs 16 MiB/core; gathers 32 MiB -> 40 MiB
      total vs 48 fp16). OSCALE and the +0.5 rounding bias fold into
      the host-side gate-coeff table since softmax weights sum to 1;
      host dequantizes with a single multiply.
    - Softmax+gate-coeff collapse runs on device (ACT exp + DVE
      reduces, fp32) from the raw weight shard.
    - Stores ride the HWDGE sync queue (plain u8, decoupled from the
      compute pipeline via 6 rotating u8 buffers); gathers keep the
      SWDGE ring to themselves. Measured HW exec ~138 us/core (vs 144
      us fp16-store baseline; HBM-bound window ~94 us + ~21 us ramp +
      compute tail).

Variants measured and rejected: SWDGE store-with-cast (ring serializes
with gathers, 175 us), two SWDGE gather queues (278 GB/s combined vs
372 single, 161 us), mixed fp16/u8 stores (sync-queue convoying stalls
the o-buffer pipeline, 172 us), dropping same-engine semaphore
waits/incs (event scheduling degrades, 162 us).
"""

import os
import sys

import numpy as np

sys.path.insert(0, "/opt/trn_rl_repo")

BATCH = 4096
IN_DIM = 8192
OUT_DIM = 16384
N_CORES = 8
J_SHARD = OUT_DIM // N_CORES
CHUNK = 128
N_CHUNKS = J_SHARD // CHUNK
NG = 4
NT = 2
NH = 3
NO = 4
NO8 = 6
CLAG = 2
DVE_PRE = 14

GATE_COEFFS = np.array([
    [0, 0, 0, 0], [0, 0, 0, 1], [0, 1, 0, -1], [0, 1, 0, 0],
    [0, 0, 1, -1], [0, 0, 1, 0], [0, 1, 1, -2], [0, 1, 1, -1],
    [1, -1, -1, 1], [1, -1, -1, 2], [1, 0, -1, 0], [1, 0, -1, 1],
    [1, -1, 0, 0], [1, -1, 0, 1], [1, 0, 0, -1], [1, 0, 0, 0],
], dtype=np.float32)

OSCALE = 254.0

# The last chunks store g as fp16 directly (no u8 cvt op): their cvt would
# sit on the critical tail after the final gather, and they have no
# downstream consumers that could convoy on the store.
FP16_TAIL = (14, 15)


def cvt_owner(j):
    return 'dve' if j % 3 == 2 else 'act'


_CACHE = {}
LAST_RESULT = None


def _wrap_idx16(idx_pair):
    cols = []
    for c in range(N_CHUNKS):
        merged = np.concatenate([idx_pair[0, c * CHUNK:(c + 1) * CHUNK],
                                 idx_pair[1, c * CHUNK:(c + 1) * CHUNK]])
        cols.append(merged.astype(np.int16).reshape(16, 16).T)
    blk = np.concatenate(cols, axis=1)
    return np.ascontiguousarray(np.tile(blk, (8, 1)))


def _build_program():
    import concourse.bacc as bacc
    import concourse.mybir as mybir
    from concourse.library_config import mlp
    from contextlib import ExitStack

    dt = mybir.dt
    AF = mybir.ActivationFunctionType

    nc = bacc.Bacc("TRN2", target_bir_lowering=False, debug=False)

    xt = nc.dram_tensor("xt", [IN_DIM, BATCH], dt.float16, kind="ExternalInput")
    idx = nc.dram_tensor("idx", [128, 2 * (J_SHARD // 16)], dt.int16,
                         kind="ExternalInput")
    wgt = nc.dram_tensor("wgt", [128, N_CHUNKS * 16], dt.float32,
                         kind="ExternalInput")
    gcr = nc.dram_tensor("gcr", [128, 4 * N_CHUNKS * 16], dt.float32,
                         kind="ExternalInput")
    out = nc.dram_tensor("out", [J_SHARD, BATCH], dt.uint8,
                         kind="ExternalOutput")
    out16 = nc.dram_tensor("out16", [len(FP16_TAIL) * CHUNK, BATCH],
                           dt.float16, kind="ExternalOutput")

    W16 = N_CHUNKS * 16

    with ExitStack() as ctx:
        sb = lambda name, shape, dty: ctx.enter_context(
            nc.sbuf_tensor(name, shape, dty))
        sb_idx = sb("sb_idx", [128, 2 * (J_SHARD // 16)], dt.int16)
        sb_w = sb("sb_w", [128, W16], dt.float32)
        sb_gc = sb("sb_gc", [128, 4 * W16], dt.float32)
        sb_e = sb("sb_e", [128, W16], dt.float32)
        sb_scr = sb("sb_scr", [128, W16], dt.float32)
        sb_s = sb("sb_s", [128, N_CHUNKS], dt.float32)
        sb_r = sb("sb_r", [128, N_CHUNKS], dt.float32)
        sb_cc = sb("sb_cc", [128, 4 * N_CHUNKS], dt.float32)
        ab_bufs = [sb(f"ab{k}", [128, 2, BATCH], dt.float16) for k in range(NG)]
        t_bufs = [sb(f"t{k}", [128, BATCH], dt.float16) for k in range(NT)]
        h_bufs = [sb(f"h{k}", [128, BATCH], dt.float16) for k in range(NH)]
        o_bufs = [sb(f"o{k}", [128, BATCH], dt.float16) for k in range(NO)]
        o8_bufs = [sb(f"q{k}", [128, BATCH], dt.uint8) for k in range(NO8)]
        of_bufs = [sb(f"f{k}", [128, BATCH], dt.float16)
                   for k in range(len(FP16_TAIL))]

        ops_act = []
        for i in range(N_CHUNKS):
            ops_act.append(('h', i))
            j = i - CLAG
            if j >= 0 and j not in FP16_TAIL and cvt_owner(j) == 'act':
                ops_act.append(('cvt', j))
        for j in range(N_CHUNKS - CLAG, N_CHUNKS):
            if j not in FP16_TAIL and cvt_owner(j) == 'act':
                ops_act.append(('cvt', j))
        act_val = {op: n + 1 for n, op in enumerate(ops_act)}

        ops_dve = []
        for i in range(N_CHUNKS):
            ops_dve.append(('ts', i))
            if i > 0:
                ops_dve.append(('add', i - 1))
            j = i - CLAG
            if j >= 0 and j not in FP16_TAIL and cvt_owner(j) == 'dve':
                ops_dve.append(('cvt', j))
            ops_dve.append(('mul', i))
        ops_dve.append(('add', N_CHUNKS - 1))
        for j in range(N_CHUNKS - CLAG, N_CHUNKS):
            if j not in FP16_TAIL and cvt_owner(j) == 'dve':
                ops_dve.append(('cvt', j))
        dve_val = {op: DVE_PRE + n + 1 for n, op in enumerate(ops_dve)}

        # Chunk 0's gather is split in two (b rows first, then a rows) so
        # DVE/ACT start ~5us earlier; its sem therefore gets 32 incs.
        def g_done(i):
            """s_g[i % NG] threshold once chunk i's gather(s) completed."""
            return 16 * (i // NG + 1) + (16 if i % NG == 0 else 0)

        def cvt_done_wait(eng, j):
            if cvt_owner(j) == 'act':
                eng.wait_ge(s_act, act_val[('cvt', j)])
            else:
                eng.wait_ge(s_dve, dve_val[('cvt', j)])

        with (
            nc.Block() as block,
            nc.semaphore("s_pi") as s_pi,
            nc.semaphore("s_pw") as s_pw,
            nc.semaphore("s_pg") as s_pg,
            nc.semaphore("s_exp") as s_exp,
            nc.semaphore("s_g0") as s_g0,
            nc.semaphore("s_g1") as s_g1,
            nc.semaphore("s_g2") as s_g2,
            nc.semaphore("s_g3") as s_g3,
            nc.semaphore("s_st0") as s_st0,
            nc.semaphore("s_st1") as s_st1,
            nc.semaphore("s_st2") as s_st2,
            nc.semaphore("s_st3") as s_st3,
            nc.semaphore("s_st4") as s_st4,
            nc.semaphore("s_st5") as s_st5,
            nc.semaphore("s_act") as s_act,
            nc.semaphore("s_dve") as s_dve,
        ):
            s_g = [s_g0, s_g1, s_g2, s_g3]
            s_st = [s_st0, s_st1, s_st2, s_st3, s_st4, s_st5]

            def cseg(k, i):
                return sb_cc[:, 16 * k + i : 16 * k + i + 1]

            @block.sync
            def _(sync):
                sync.dma_start(sb_idx[:, :], idx[:, :]).then_inc(s_pi, 16)
                sync.dma_start(sb_w[:, :], wgt[:, :]).then_inc(s_pw, 16)
                sync.dma_start(sb_gc[:, :], gcr[:, :]).then_inc(s_pg, 16)
                for i in range(N_CHUNKS):
                    ks = i % NO8
                    if i in FP16_TAIL:
                        sync.wait_ge(s_dve, dve_val[('add', i)])
                        r0 = FP16_TAIL.index(i) * CHUNK
                        sync.dma_start(out16[r0:r0 + CHUNK, :],
                                       of_bufs[FP16_TAIL.index(i)][:, :],
                                       ).then_inc(s_st[ks], 16)
                    else:
                        cvt_done_wait(sync, i)
                        sync.dma_start(out[i * CHUNK:(i + 1) * CHUNK, :],
                                       o8_bufs[ks][:, :]).then_inc(s_st[ks], 16)
                for ks in range(NO8):
                    n_st = (N_CHUNKS - 1 - ks) // NO8 + 1
                    sync.wait_ge(s_st[ks], 16 * n_st)

            @block.gpsimd
            def _(gp):
                gp.load_library(mlp)
                nreg = gp.alloc_register("nidx")
                gp.reg_mov(nreg, 2 * CHUNK)
                nreg1 = gp.alloc_register("nidx1")
                gp.reg_mov(nreg1, CHUNK)
                gp.wait_ge(s_pi, 16)
                # Chunk 0 split: b rows first (ts/h consume b), then a rows.
                # Index i of the wrapped chunk lives at [i%16, i//16], so
                # the b half (indices 128..255) is cols 8..16.
                gp.dma_gather(
                    ab_bufs[0][:, 1:2, :], xt.ap(),
                    sb_idx[:, 8:16], CHUNK, nreg1, BATCH,
                ).then_inc(s_g[0], 16)
                gp.dma_gather(
                    ab_bufs[0][:, 0:1, :], xt.ap(),
                    sb_idx[:, 0:8], CHUNK, nreg1, BATCH,
                ).then_inc(s_g[0], 16)
                for i in range(1, N_CHUNKS):
                    kg = i % NG
                    if i >= NG:
                        gp.wait_ge(s_dve, dve_val[('mul', i - NG)])
                        gp.wait_ge(s_act, act_val[('h', i - NG)])
                        gp.wait_ge(s_g[kg], g_done(i - NG))
                    gp.dma_gather(
                        ab_bufs[kg].ap(), xt.ap(),
                        sb_idx[:, 16 * i:16 * i + 16], 2 * CHUNK, nreg, BATCH,
                    ).then_inc(s_g[kg], 16)

            @block.scalar
            def _(sc):
                sc.wait_ge(s_pw, 16)
                sc.activation(sb_e[:, :], sb_w[:, :], AF.Exp).then_inc(s_exp, 1)
                sc.wait_ge(s_dve, DVE_PRE)
                for kind, i in ops_act:
                    if kind == 'h':
                        k = i % NH
                        kg = i % NG
                        # h reads b only; chunk 0's b half is the first inc
                        sc.wait_ge(s_g[kg], 16 if i == 0 else g_done(i))
                        if i >= NH:
                            sc.wait_ge(s_dve, dve_val[('add', i - NH)])
                        sc.activation(h_bufs[k][:, :], ab_bufs[kg][:, 1, :],
                                      AF.Identity,
                                      bias=cseg(0, i), scale=cseg(2, i),
                                      ).then_inc(s_act, 1)
                    else:
                        ko, ks = i % NO, i % NO8
                        sc.wait_ge(s_dve, dve_val[('add', i)])
                        if i >= NO8:
                            sc.wait_ge(s_st[ks], 16 * (i // NO8))
                        sc.activation(o8_bufs[ks][:, :], o_bufs[ko][:, :],
                                      AF.Identity,
                                      ).then_inc(s_act, 1)

            @block.vector
            def _(v):
                X = mybir.AxisListType.X
                n = 0

                def step(ins):
                    nonlocal n
                    n += 1
                    ins.then_inc(s_dve, 1)

                v.wait_ge(s_exp, 1)
                v.wait_ge(s_pg, 16)
                e3 = sb_e[:, :].rearrange("p (c g) -> p c g", g=16)
                step(v.reduce_sum(sb_s[:, :], e3, axis=X))
                v.wait_ge(s_dve, n)
                step(v.reciprocal(sb_r[:, :], sb_s[:, :]))
                for kk in range(4):
                    if kk > 0:
                        v.wait_ge(s_dve, n)
                    step(v.tensor_mul(sb_scr[:, :], sb_e[:, :],
                                      sb_gc[:, kk * W16:(kk + 1) * W16]))
                    v.wait_ge(s_dve, n)
                    step(v.reduce_sum(
                        sb_cc[:, 16 * kk:16 * (kk + 1)],
                        sb_scr[:, :].rearrange("p (c g) -> p c g", g=16),
                        axis=X))
                v.wait_ge(s_dve, n)
                for kk in range(4):
                    step(v.tensor_mul(sb_cc[:, 16 * kk:16 * (kk + 1)],
                                      sb_cc[:, 16 * kk:16 * (kk + 1)],
                                      sb_r[:, :]))
                assert n == DVE_PRE
                v.wait_ge(s_dve, DVE_PRE)
                MU, AD = mybir.AluOpType.mult, mybir.AluOpType.add
                for kind, i in ops_dve:
                    kt, kh, ko, kg = i % NT, i % NH, i % NO, i % NG
                    ks = i % NO8
                    if kind == 'ts':
                        # ts reads b only; chunk 0's b half is the first inc
                        v.wait_ge(s_g[kg], 16 if i == 0 else g_done(i))
                        if i >= NT:
                            v.wait_ge(s_dve, dve_val[('mul', i - NT)])
                        v.tensor_scalar(t_bufs[kt][:, :], ab_bufs[kg][:, 1, :],
                                        cseg(3, i), cseg(1, i), MU, AD,
                                        ).then_inc(s_dve, 1)
                    elif kind == 'mul':
                        v.wait_ge(s_dve, dve_val[('ts', i)])
                        if i == 0:
                            v.wait_ge(s_g[0], 32)  # chunk 0's a half landed
                        if i >= NO:
                            j = i - NO
                            if cvt_owner(j) == 'act':
                                v.wait_ge(s_act, act_val[('cvt', j)])
                        v.tensor_mul(o_bufs[ko][:, :], t_bufs[kt][:, :],
                                     ab_bufs[kg][:, 0, :]).then_inc(s_dve, 1)
                    elif kind == 'add':
                        v.wait_ge(s_act, act_val[('h', i)])
                        v.wait_ge(s_dve, dve_val[('mul', i)])
                        dst = (of_bufs[FP16_TAIL.index(i)] if i in FP16_TAIL
                               else o_bufs[ko])
                        v.tensor_add(dst[:, :], o_bufs[ko][:, :],
                                     h_bufs[kh][:, :]).then_inc(s_dve, 1)
                    else:
                        if i >= NO8:
                            v.wait_ge(s_st[ks], 16 * (i // NO8))
                        v.tensor_copy(o8_bufs[ks][:, :],
                                      o_bufs[ko][:, :]).then_inc(s_dve, 1)

    nc.compile()
    return nc


def _get_program():
    if "nc" not in _CACHE:
        _CACHE["nc"] = _build_program()
    return _CACHE["nc"]


def kernel(x, weight, indices):
    global LAST_RESULT
    from concourse.bass_utils import run_bass_kernel_spmd

    x = np.asarray(x, dtype=np.float32)
    weight = np.asarray(weight, dtype=np.float32)
    indices = np.asarray(indices)

    nc = _get_program()

    xt = np.ascontiguousarray(x.T.astype(np.float16))

    gate_scaled = GATE_COEFFS * OSCALE
    gate_scaled[:, 0] += 0.5
    gc_rep = np.broadcast_to(
        gate_scaled.T.reshape(4, 1, 16),
        (4, N_CHUNKS, 16)).reshape(1, -1)
    gc_rep = np.ascontiguousarray(
        np.broadcast_to(gc_rep, (128, 4 * N_CHUNKS * 16)).astype(np.float32))

    in_maps = []
    for c in range(N_CORES):
        j0 = c * J_SHARD
        idx_c = _wrap_idx16(indices[:, j0:j0 + J_SHARD])
        wsh = weight[j0:j0 + J_SHARD]
        w_wrapped = np.ascontiguousarray(
            wsh.reshape(N_CHUNKS, 128, 16).transpose(1, 0, 2)
            .reshape(128, N_CHUNKS * 16))
        in_maps.append({
            "xt": xt,
            "idx": idx_c,
            "wgt": w_wrapped,
            "gcr": gc_rep,
        })

    trace = bool(os.environ.get("KERNEL_TRACE"))
    res = run_bass_kernel_spmd(nc, in_maps, core_ids=list(range(N_CORES)),
                               trace=trace)
    LAST_RESULT = res

    inv = np.float32(1.0 / OSCALE)
    shards = []
    for c in range(N_CORES):
        full = res.results[c]["out"].astype(np.float32) * inv
        g16 = res.results[c]["out16"]            # fp16 rows of FP16_TAIL
        for n, i in enumerate(FP16_TAIL):
            blk = g16[n * CHUNK:(n + 1) * CHUNK, :].astype(np.float32)
            full[i * CHUNK:(i + 1) * CHUNK, :] = (blk - 0.5) * inv
        shards.append(full)
    full = np.concatenate(shards, axis=0)
    return np.ascontiguousarray(full.T)
